# revision 7
# baseline (speedup 1.0000x reference)
"""Heat equation (512x512, 399 output steps) on 8 trn2 NeuronCores.

v6: fp16 state/compute. Host computes step 1 (f32); device computes steps
2..399. Sharding: 1D row strips, 64 owned rows/core, 32-row halo each side.
Halo refreshed via fp16 AllGather every 32 steps.

The 512 interior columns are split into two SEPARATE state tiles (the Tile
framework tracks dependencies per tile, so a single state tile serializes
every step behind its last write):
  stL [128,258]: col0 = zero guard | cols1..256 = interior 0..255 |
                 col257 = copy of interior 256 (seam, written by R chain)
  stR [128,258]: col0 = copy of interior 255 (seam, written by L chain) |
                 cols1..256 = interior 256..511 | col257 = zero guard
Per chunk X: PE computes the 5-point laplacian into PSUM with two matmuls
(wud tridiag center + one ident "combo" matmul whose stride-0 output AP
visits each PSUM column twice, accumulating both shifted windows); DVE does
m = A o psum (f32 read) and Tn = Tp + m, plus a 1-column seam add into the
other tile. A = DT*dmap/DX2 (f32, zeroed on the global boundary) keeps
Dirichlet rows/cols frozen at zero.
"""
import numpy as np

N = 512
NCORES = 8
ROWS = 64          # owned rows per core
H = 32             # halo depth
DSTEPS = 398       # device computes T_2..T_399
PERIOD = 32        # halo exchange period
DT = 5e-7
DX = 1.0 / (N - 1)
DX2 = DX * DX
HALF = N // 2      # 256

_prog_cache = {}


def _build_program(steps, period):
    from concourse import bass, bacc, tile, mybir
    import bass_rust

    F16 = mybir.dt.float16
    F32 = mybir.dt.float32
    nc = bacc.Bacc("TRN2", target_bir_lowering=False, debug=False,
                   num_devices=NCORES)

    tl_in = nc.dram_tensor("t1l", [128, HALF + 2], F16, kind="ExternalInput")
    tr_in = nc.dram_tensor("t1r", [128, HALF + 2], F16, kind="ExternalInput")
    a_in = nc.dram_tensor("amap", [128, N], F32, kind="ExternalInput")
    w_in = nc.dram_tensor("wud", [128, 128], F16, kind="ExternalInput")
    i_in = nc.dram_tensor("ident", [128, 128], F16, kind="ExternalInput")
    z_in = nc.dram_tensor("zg", [128, 2], F16, kind="ExternalInput")
    out = nc.dram_tensor("out", [steps, ROWS, N], F16, kind="ExternalOutput")
    in_bounce = nc.dram_tensor("in_bounce", [ROWS, N], F16)
    ag_out = nc.dram_tensor("ag_out", [NCORES * ROWS, N], F16,
                            addr_space="Shared")

    add = mybir.AluOpType.add
    mult = mybir.AluOpType.mult

    with tile.TileContext(nc) as tc:
        with tc.tile_pool(name="state", bufs=1) as spool, \
             tc.tile_pool(name="consts", bufs=1) as cpool, \
             tc.tile_pool(name="psum", bufs=4, space="PSUM") as ppool, \
             tc.tile_pool(name="scratch", bufs=3) as zpool:
            NB = 8
            stL = [spool.tile([128, HALF + 2], F16, tag=f"stL{i}",
                              name=f"stL{i}") for i in range(NB)]
            stR = [spool.tile([128, HALF + 2], F16, tag=f"stR{i}",
                              name=f"stR{i}") for i in range(NB)]
            amap = cpool.tile([128, N], F32, tag="amap")
            wud = cpool.tile([128, 128], F16, tag="wud")
            ident = cpool.tile([128, 128], F16, tag="ident")

            nc.sync.dma_start(out=amap[:], in_=a_in[:])
            nc.sync.dma_start(out=wud[:], in_=w_in[:])
            nc.sync.dma_start(out=ident[:], in_=i_in[:])
            nc.sync.dma_start(out=stL[0][:], in_=tl_in[:])
            nc.sync.dma_start(out=stR[0][:], in_=tr_in[:])
            for i in range(1, NB):
                nc.gpsimd.dma_start(out=stL[i][:, 0:1], in_=z_in[:, 0:1])
                nc.gpsimd.dma_start(out=stR[i][:, HALF + 1:HALF + 2],
                                    in_=z_in[:, 1:2])

            r = nc.gpsimd.partition_id()
            ofs_top = nc.s_assert_within(r * ROWS - H, 0, NCORES * ROWS - H,
                                         skip_runtime_assert=True)
            ofs_bot = nc.s_assert_within(r * ROWS + ROWS, 0,
                                         NCORES * ROWS - H,
                                         skip_runtime_assert=True)

            dma_engines = [nc.sync, nc.gpsimd, nc.scalar]

            def combo_rhs(src):
                rhs = src[:, 0:HALF].copy()
                part = tuple(rhs.ap.to_list()[0])
                rhs.ap = bass_rust.VecI64Pair([part, (2, 2), (1, HALF)])
                return rhs

            for k in range(steps):
                TpL, TnL = stL[k % NB], stL[(k + 1) % NB]
                TpR, TnR = stR[k % NB], stR[(k + 1) % NB]
                psL = ppool.tile([128, HALF], F32, tag="psL")
                psR = ppool.tile([128, HALF], F32, tag="psR")
                mL = zpool.tile([128, HALF], F16, tag="mL")
                mR = zpool.tile([128, HALF], F16, tag="mR")

                nc.tensor.matmul(psL[:], wud[:], TpL[:, 1:HALF + 1],
                                 start=True, stop=False)
                nc.tensor.matmul(psL[:].unsqueeze(1).broadcast_to(
                                     [128, 2, HALF]),
                                 ident[:], combo_rhs(TpL),
                                 start=False, stop=True, skip_group_check=True)
                nc.tensor.matmul(psR[:].unsqueeze(1).broadcast_to(
                                     [128, 2, HALF]),
                                 ident[:], combo_rhs(TpR),
                                 start=True, stop=False, skip_group_check=True)
                nc.tensor.matmul(psR[:], wud[:], TpR[:, 1:HALF + 1],
                                 start=False, stop=True, skip_group_check=True)

                nc.vector.tensor_tensor(mL[:], amap[:, 0:HALF], psL[:], mult)
                nc.vector.tensor_tensor(TnL[:, 1:HALF + 1],
                                        TpL[:, 1:HALF + 1], mL[:], add)
                # seam: interior 255 into TnR col 0
                nc.vector.tensor_tensor(TnR[:, 0:1],
                                        TpL[:, HALF:HALF + 1],
                                        mL[:, HALF - 1:HALF], add)
                nc.vector.tensor_tensor(mR[:], amap[:, HALF:N], psR[:], mult)
                # seam: interior 256 into TnL col 257
                nc.vector.tensor_tensor(TnL[:, HALF + 1:HALF + 2],
                                        TpR[:, 1:2], mR[:, 0:1], add)
                nc.vector.tensor_tensor(TnR[:, 1:HALF + 1],
                                        TpR[:, 1:HALF + 1], mR[:], add)

                eng = dma_engines[k % len(dma_engines)]
                eng.dma_start(out=out[k][:, 0:HALF],
                              in_=TnL[H:H + ROWS, 1:HALF + 1])
                eng2 = dma_engines[(k + 1) % len(dma_engines)]
                eng2.dma_start(out=out[k][:, HALF:N],
                               in_=TnR[H:H + ROWS, 1:HALF + 1])

                if (k + 2) % period == 0 and k < steps - 14:
                    nc.sync.dma_start(out=in_bounce[:, 0:HALF],
                                      in_=TnL[H:H + ROWS, 1:HALF + 1])
                    nc.sync.dma_start(out=in_bounce[:, HALF:N],
                                      in_=TnR[H:H + ROWS, 1:HALF + 1])
                    nc.gpsimd.collective_compute(
                        "AllGather",
                        mybir.AluOpType.bypass,
                        replica_groups=[list(range(NCORES))],
                        ins=[in_bounce[:]],
                        outs=[ag_out[:]],
                    )
                    for ofs, p0, p1 in ((ofs_top, 0, H),
                                        (ofs_bot, H + ROWS, 128)):
                        nc.gpsimd.dma_start(
                            out=TnL[p0:p1, 1:HALF + 2],
                            in_=ag_out[bass.ds(ofs, H), 0:HALF + 1],
                            bounds_check="skip_entire_dma")
                        nc.gpsimd.dma_start(
                            out=TnR[p0:p1, 0:HALF + 1],
                            in_=ag_out[bass.ds(ofs, H), HALF - 1:N],
                            bounds_check="skip_entire_dma")

    nc.compile()
    return nc


def _bilinear_f32(a, out_h, out_w):
    """numpy float32 mirror of reference bilinear_align_corners."""
    in_h, in_w = a.shape
    ys = np.linspace(0.0, in_h - 1.0, out_h, dtype=np.float32)
    xs = np.linspace(0.0, in_w - 1.0, out_w, dtype=np.float32)
    y0 = np.clip(np.floor(ys).astype(np.int32), 0, in_h - 2)
    x0 = np.clip(np.floor(xs).astype(np.int32), 0, in_w - 2)
    wy = (ys - y0.astype(np.float32))[:, None]
    wx = (xs - x0.astype(np.float32))[None, :]
    a00 = a[y0][:, x0]
    a01 = a[y0][:, x0 + 1]
    a10 = a[y0 + 1][:, x0]
    a11 = a[y0 + 1][:, x0 + 1]
    return (a00 * (1 - wy) * (1 - wx) + a01 * (1 - wy) * wx
            + a10 * wy * (1 - wx) + a11 * wy * wx).astype(np.float32)


def kernel(u0, alpha, steps=DSTEPS, period=PERIOD):
    from concourse.bass_utils import run_bass_kernel_spmd

    u0 = np.asarray(u0, dtype=np.float32)
    alpha = np.asarray(alpha, dtype=np.float32)

    dmap = _bilinear_f32(alpha, N, N)
    a_in = dmap[1:-1, 1:-1]

    # host computes step 1 exactly as the f32 reference does
    lap = (u0[:-2, 1:-1] - 2.0 * u0[1:-1, 1:-1] + u0[2:, 1:-1]
           + u0[1:-1, :-2] - 2.0 * u0[1:-1, 1:-1] + u0[1:-1, 2:]) / np.float32(DX2)
    inner = u0[1:-1, 1:-1] + np.float32(DT) * a_in * lap
    T1 = np.zeros((N, N), np.float32)
    T1[1:-1, 1:-1] = inner

    A = (np.float32(DT) * dmap / np.float32(DX2)).astype(np.float32)
    A[0, :] = 0.0
    A[N - 1, :] = 0.0
    A[:, 0] = 0.0
    A[:, N - 1] = 0.0

    T1h = T1.astype(np.float16)

    wud = np.zeros((128, 128), np.float16)
    for m in range(128):
        wud[m, m] = -4.0
        if m > 0:
            wud[m - 1, m] = 1.0
        if m < 127:
            wud[m + 1, m] = 1.0
    ident = np.eye(128, dtype=np.float16)

    in_maps = []
    for i in range(NCORES):
        lo = i * ROWS - H          # global row of tile partition 0
        g0, g1 = max(lo, 0), min(lo + 128, N)
        t1l = np.zeros((128, HALF + 2), np.float16)
        t1r = np.zeros((128, HALF + 2), np.float16)
        t1l[g0 - lo:g1 - lo, 1:HALF + 1] = T1h[g0:g1, 0:HALF]
        t1l[g0 - lo:g1 - lo, HALF + 1] = T1h[g0:g1, HALF]
        t1r[g0 - lo:g1 - lo, 0] = T1h[g0:g1, HALF - 1]
        t1r[g0 - lo:g1 - lo, 1:HALF + 1] = T1h[g0:g1, HALF:N]
        at = np.zeros((128, N), np.float32)
        at[g0 - lo:g1 - lo] = A[g0:g1]
        in_maps.append({"t1l": t1l, "t1r": t1r, "amap": at,
                        "wud": wud, "ident": ident,
                        "zg": np.zeros((128, 2), np.float16)})

    key = (steps, period)
    if key not in _prog_cache:
        _prog_cache[key] = _build_program(steps, period)
    nc = _prog_cache[key]

    res = run_bass_kernel_spmd(nc, in_maps, list(range(NCORES)))
    globals()["_last_res"] = res
    full = np.empty((steps + 1, N, N), np.float32)
    full[0] = T1
    dev = np.concatenate([res.results[i]["out"] for i in range(NCORES)],
                         axis=1)
    full[1:] = dev.astype(np.float32)
    return full
